# revision 27
# baseline (speedup 1.0000x reference)
"""Multi-head attention (B=2, L=2048, H=16, D=64) on 8 TRN2 NeuronCores.

Sharding: core = (batch b, head-group hg); 2 batches x 4 groups of 4 heads.
Per core, for its batch and its 4 heads (2 head-pairs m, heads hl in pair):
    Q^T/K^T = W^T x^T            (d on partitions; head hl of pair m at
                                  rows 64*hl : 64*hl+64)
    V       = x W_v              (j on partitions)
    S^T     = K^T.T Q^T          (j on partitions, i free; row-tiled: the
                                  two heads' matmuls run concurrently on
                                  disjoint 64-row groups of the PE array)
    P'      = exp(S^T/8)         (fp16, un-normalized softmax numerator)
    esum   += P'                 (DVE fp16; partial denominator per j-row)
    O'^T   += V.T P'             (col-tiled: head hl accumulates into
                                  partitions 64*hl:64*hl+64 of ONE psum
                                  tile, so both heads run concurrently)
    D       = 1^T esum           (tiny matmul: denominator per column i)
    O^T     = O'^T * (1/D)       (DVE fp16, broadcast 1/D across rows)
    out^T  += Wo_rows^T O^T      (partial over head-group rows of Wo)
Host sums the 4 partials per batch, transposes, adds bo.

All matmul operands are float16 (1 cyc/row on the PE at 2.4 GHz warm vs
~2x slower for f32r; PSUM accumulation stays f32).  The attention j-loop
is ACT(exp)-bound at ~2.3us per j; the schedule keeps the scalar engine
saturated:

  - PSUM: 4 banks S ping/pong (pool ps) + 2 banks merged O' accumulator
    (pool po) + 2 banks spare (pool px).  The spare pool absorbs V/QK
    projection tiles, Wo chunks and denominator matmuls WITHOUT touching
    the S ping/pong, so interleaved work no longer stalls the exp pipe.
  - x^T is DMA'd in [128,1024] chunks, low seq-half on the sync queue,
    high half concurrently on the gpsimd queue; Q/K first-half
    projections run k-chunk-major across 4 PSUM tiles tracking DMA
    arrival; dummy matmuls on Wq warm the PE HAM clock-gate first.
  - unit order (ih,m) = (0,0),(1,0),(0,1),(1,1): V j-tiles 2-15 stream
    into unit 0's loop, the m=1 projections into units 0-1, the ih=0
    output projection into unit 3.  Only unit 3's normalize + the ih=1
    output projection (direct PSUM->DRAM stores) remain as the tail.
  - each unit's softmax normalization is deferred into the next unit's
    j-loop (reciprocal chain hides under the exp stream).
"""

import sys

try:
    import concourse.bass as bass  # noqa: F401
except ImportError:  # pragma: no cover - path fallback
    sys.path.insert(0, "/opt/trn_rl_repo")

import numpy as np
import concourse.bass as bass
import concourse.mybir as mybir
import concourse.tile as tile
from concourse import bacc
from concourse.bass_utils import run_bass_kernel_spmd

F32 = mybir.dt.float32
F16 = mybir.dt.float16
AF = mybir.ActivationFunctionType

B = 2
L = 2048          # sequence length
C = 1024          # model dim
H_LOC = 4         # heads per core
D = 64            # head dim
HD = H_LOC * D    # 256 = local head-group width
KT = C // 128     # 8 k-tiles over the model dim
SCALE2 = float(D) ** -0.5  # 1/8, applied once inside exp

_cache = {}


def _build():
    nc = bacc.Bacc("TRN2", target_bir_lowering=False, debug=False, num_devices=8)

    xT = nc.declare_dram_parameter("xT", [C, L], F16, isOutput=False)
    wq = nc.declare_dram_parameter("wq", [C, HD], F16, isOutput=False)
    wk = nc.declare_dram_parameter("wk", [C, HD], F16, isOutput=False)
    wv = nc.declare_dram_parameter("wv", [C, HD], F16, isOutput=False)
    wo = nc.declare_dram_parameter("wo", [HD, C], F16, isOutput=False)
    outT = nc.declare_dram_parameter("outT", [C, L], F32, isOutput=True)

    with tile.TileContext(nc) as tc:
        with tc.tile_pool(name="sb", bufs=1) as sb, \
             tc.tile_pool(name="ps", bufs=2, space="PSUM") as ps, \
             tc.tile_pool(name="po", bufs=1, space="PSUM") as po, \
             tc.tile_pool(name="px", bufs=1, space="PSUM") as px:

            def lp(reason="fp16 compute"):
                return nc.allow_low_precision(reason=reason)

            # ---- input DMA: weights + low seq-half on sync queue, high
            # half concurrently on the gpsimd queue ------------------------
            wq_sb = sb.tile([128, KT, HD], F16, tag="wq")
            wk_sb = sb.tile([128, KT, HD], F16, tag="wk")
            wv_sb = sb.tile([128, KT, HD], F16, tag="wv")
            wo_sb = sb.tile([128, 2, C], F16, tag="wo")
            xT_sb = sb.tile([128, KT, L], F16, tag="xT")
            nc.sync.dma_start(wq_sb[:, :, :], wq.rearrange("(k p) c -> p k c", p=128))
            nc.sync.dma_start(wk_sb[:, :, :], wk.rearrange("(k p) c -> p k c", p=128))
            nc.sync.dma_start(wv_sb[:, :, :], wv.rearrange("(k p) c -> p k c", p=128))
            for k in range(KT):
                nc.sync.dma_start(xT_sb[:, k, 0:1024],
                                  xT[k * 128:(k + 1) * 128, 0:1024])
            for k in range(KT):
                nc.gpsimd.dma_start(xT_sb[:, k, 1024:2048],
                                    xT[k * 128:(k + 1) * 128, 1024:2048])
            nc.gpsimd.dma_start(wo_sb[:, :, :], wo.rearrange("(k p) c -> p k c", p=128))

            ones16 = sb.tile([128, D], F16, tag="ones16")
            with lp():
                nc.vector.memset(ones16[:], 1.0)
            warm_junk = sb.tile([128, 8], F32, tag="warm_junk")

            qT_sb = sb.tile([128, 2, L], F16, tag="qT")
            kT_sb = sb.tile([128, 2, L], F16, tag="kT")
            v_sb = sb.tile([128, 16, H_LOC, D], F16, tag="v")
            oT_sb = sb.tile([128, 2, L], F16, tag="oT")
            esum_sb = sb.tile([128, 2, 1024], F16, tag="esum")

            def emit_warm(pool, nmm=6):
                pd = pool.tile([128, 1024], F32, tag=pool.name[1], name="warm")
                for i in range(nmm):
                    nc.tensor.matmul(
                        pd[:, 0:HD],
                        wq_sb[:, 0, 0:128],
                        wq_sb[:, 0, :],
                        start=(i == 0), stop=(i == nmm - 1),
                    )
                nc.vector.tensor_copy(warm_junk[:], pd[:, 0:8])

            # ---- PE warm-up: dummy matmuls on wq keep the HAM clock-gate
            # busy while the x^T DMA streams in ----------------------------
            for g in range(3):
                emit_warm(ps)

            def copy16(dst, src):
                with lp():
                    nc.vector.tensor_copy(dst, src)

            # ---- Q/K m=0 projections, seq-half 0: k-chunk-major across 4
            # PSUM tiles so the PE keeps pace with the x^T chunk arrivals --
            pk0 = ps.tile([128, 1024], F32, tag="s", name="pk0")
            pk1 = ps.tile([128, 1024], F32, tag="s", name="pk1")
            pq0 = po.tile([128, 1024], F32, tag="o", name="pq0")
            pq1 = px.tile([128, 1024], F32, tag="x", name="pq1")
            for k in range(KT):
                for acc, w_sb, n in ((pk0, wk_sb, 0), (pk1, wk_sb, 1),
                                     (pq0, wq_sb, 0), (pq1, wq_sb, 1)):
                    nc.tensor.matmul(
                        acc[:, 0:512],
                        w_sb[:, k, 0:128],
                        xT_sb[:, k, n * 512:(n + 1) * 512],
                        start=(k == 0), stop=(k == KT - 1),
                    )
            copy16(kT_sb[:, 0, 0:512], pk0[:, 0:512])
            copy16(kT_sb[:, 0, 512:1024], pk1[:, 0:512])
            copy16(qT_sb[:, 0, 0:512], pq0[:, 0:512])
            copy16(qT_sb[:, 0, 512:1024], pq1[:, 0:512])

            # K^T m=0, seq-half 1: k-chunk-major across the 2 S-pool tiles
            pk2 = ps.tile([128, 1024], F32, tag="s", name="pk2")
            pk3 = ps.tile([128, 1024], F32, tag="s", name="pk3")
            for k in range(KT):
                for acc, n in ((pk2, 2), (pk3, 3)):
                    nc.tensor.matmul(
                        acc[:, 0:512],
                        wk_sb[:, k, 0:128],
                        xT_sb[:, k, n * 512:(n + 1) * 512],
                        start=(k == 0), stop=(k == KT - 1),
                    )
            copy16(kT_sb[:, 0, 1024:1536], pk2[:, 0:512])
            copy16(kT_sb[:, 0, 1536:2048], pk3[:, 0:512])

            def emit_v_tile(it, pool):
                p = pool.tile([128, 1024], F32, tag=pool.name[1], name="vp")
                acc = p[:, 0:HD]
                for k in range(KT):
                    nc.tensor.matmul(
                        acc,
                        xT_sb[:, k, it * 128:(it + 1) * 128],
                        wv_sb[:, k, :],
                        start=(k == 0), stop=(k == KT - 1),
                    )
                copy16(
                    v_sb[:, it, :, :],
                    acc.rearrange("p (h d) -> p h d", h=H_LOC),
                )

            def emit_proj_tile(w_sb, t_sb, m, n):
                p = px.tile([128, 1024], F32, tag="x", name="proj")
                acc = p[:, 0:512]
                for k in range(KT):
                    nc.tensor.matmul(
                        acc,
                        w_sb[:, k, m * 128:(m + 1) * 128],
                        xT_sb[:, k, n * 512:(n + 1) * 512],
                        start=(k == 0), stop=(k == KT - 1),
                    )
                copy16(t_sb[:, m, n * 512:(n + 1) * 512], acc)

            # first two V j-tiles before the attention loop; the rest stream
            # into unit 0's slack
            emit_v_tile(0, po)
            emit_v_tile(1, px)

            es_pool = tc.alloc_tile_pool(name="es_pool", bufs=6)
            st_pool = tc.alloc_tile_pool(name="st_pool", bufs=2)
            np_pool = tc.alloc_tile_pool(name="np_pool", bufs=4)
            ost_pool = tc.alloc_tile_pool(name="ost_pool", bufs=4)

            pending = []   # deferred normalize: (m, i0, o_cps, rep16)

            def emit_normalize():
                m, i0, o_cp, rep = pending.pop(0)
                with lp():
                    nc.vector.tensor_mul(
                        oT_sb[0:64, m, i0:i0 + 1024],
                        o_cp[0:64, :], rep[0:64, :])
                    stage = st_pool.tile([128, 1024], F16, tag="stage")
                    nc.vector.tensor_mul(
                        stage[64:128, :], o_cp[64:128, :], rep[64:128, :])
                    nc.gpsimd.dma_start(
                        oT_sb[64:128, m, i0:i0 + 1024], stage[64:128, :])

            def emit_wo_chunk(ih, ct, pool, copy_engine=None):
                # [128, 1024] output chunk staged through SBUF (DMA cannot
                # read PSUM); the staging copy engine is selectable so the
                # tail can use the then-idle scalar/gpsimd engines
                i0 = ih * 1024
                acc = pool.tile([128, 1024], F32, tag=pool.name[1], name="wo_ps")
                for kk in range(2):
                    for n in range(2):
                        nc.tensor.matmul(
                            acc[:, n * 512:(n + 1) * 512],
                            wo_sb[:, kk, ct * 128:(ct + 1) * 128],
                            oT_sb[:, kk, i0 + n * 512:i0 + (n + 1) * 512],
                            start=(kk == 0), stop=(kk == 1),
                        )
                ost = ost_pool.tile([128, 1024], F32, tag="ost", name="ost")
                if copy_engine == "scalar":
                    nc.scalar.copy(ost[:], acc[:])
                else:
                    nc.vector.tensor_copy(ost[:], acc[:])
                nc.sync.dma_start(
                    outT[ct * 128:(ct + 1) * 128, i0:i0 + 1024], ost[:])

            # per-unit interleave schedule: j-step -> list of closures, all
            # using the spare px pool (no S ping/pong contention).
            interleave = [dict() for _ in range(4)]
            interleave[0][0] = [lambda: emit_v_tile(2, px),
                                lambda: emit_v_tile(3, px)]
            interleave[0][1] = [lambda: emit_v_tile(4, px),
                                lambda: emit_v_tile(5, px)]
            for j in range(2, 12):
                interleave[0][j] = [
                    (lambda it: lambda: emit_v_tile(it, px))(j + 4)]
            interleave[0][12] = [lambda: emit_proj_tile(wq_sb, qT_sb, 0, 2)]
            interleave[0][13] = [lambda: emit_proj_tile(wq_sb, qT_sb, 0, 3)]
            interleave[1] = {
                1: [lambda: emit_proj_tile(wk_sb, kT_sb, 1, 0)],
                4: [lambda: emit_proj_tile(wq_sb, qT_sb, 1, 0)],
                7: [lambda: emit_proj_tile(wq_sb, qT_sb, 1, 1)],
                8: [emit_normalize],
                10: [lambda: emit_proj_tile(wk_sb, kT_sb, 1, 1)],
                13: [lambda: emit_proj_tile(wk_sb, kT_sb, 1, 2)],
            }
            interleave[2] = {
                1: [lambda: emit_proj_tile(wk_sb, kT_sb, 1, 3)],
                5: [lambda: emit_proj_tile(wq_sb, qT_sb, 1, 2)],
                8: [emit_normalize],
                9: [lambda: emit_proj_tile(wq_sb, qT_sb, 1, 3)],
            }
            interleave[3] = {
                2: [emit_normalize],
                3: [lambda: emit_wo_chunk(0, 0, px)],
                4: [lambda: emit_wo_chunk(0, 1, px)],
                6: [lambda: emit_wo_chunk(0, 2, px)],
                8: [lambda: emit_wo_chunk(0, 3, px)],
                10: [lambda: emit_wo_chunk(0, 4, px)],
                11: [lambda: emit_wo_chunk(0, 5, px)],
                13: [lambda: emit_wo_chunk(0, 6, px)],
                14: [lambda: emit_wo_chunk(0, 7, px)],
            }

            units = [(0, 0), (1, 0), (0, 1), (1, 1)]  # (ih, m)
            for ui, (ih, m) in enumerate(units):
                i0 = ih * 1024

                def emit_s(j):
                    s_list = []
                    for hl in range(2):
                        r0 = hl * 64
                        s_ps = ps.tile([128, 1024], F32, tag="s", name=f"s_ps{hl}")
                        for n in range(2):
                            nc.tensor.matmul(
                                s_ps[:, n * 512:(n + 1) * 512],
                                kT_sb[r0:r0 + 64, m, j * 128:(j + 1) * 128],
                                qT_sb[r0:r0 + 64, m,
                                      i0 + n * 512:i0 + (n + 1) * 512],
                                start=True, stop=True,
                            )
                        s_list.append(s_ps)
                    return s_list

                o_t = po.tile([128, 1024], F32, tag="o", name="o_ps")
                es = [None, None]
                # S is emitted one step ahead of its exp so the next step's
                # S matmuls sit in the PE queue BEFORE any interleaved task
                # matmuls (in-order engine queue: this keeps the exp pipe
                # fed even when a task overflows the per-step PE budget).
                s_list = emit_s(0)
                for j in range(16):
                    prev_es = es
                    es = []
                    for hl in range(2):
                        e_sb = es_pool.tile([128, 1024], F16, tag="es",
                                            name=f"es{hl}")
                        with lp():
                            nc.scalar.activation(e_sb[:], s_list[hl][:], AF.Exp,
                                                 scale=SCALE2)
                        es.append(e_sb)
                    if j < 15:
                        s_list = emit_s(j + 1)
                    # running denominator: esum += P' (DVE fp16 2x mode)
                    for hl in range(2):
                        with lp():
                            if j == 0:
                                nc.vector.tensor_copy(
                                    esum_sb[:, hl, :], es[hl][:])
                            else:
                                nc.vector.tensor_add(
                                    esum_sb[:, hl, :],
                                    esum_sb[:, hl, :], es[hl][:])
                    # AV for step j-1 (pipelined one step behind); the two
                    # heads are column-tiled into one merged O' accumulator
                    if j > 0:
                        for n in range(2):
                            for hl in range(2):
                                nc.tensor.matmul(
                                    o_t[hl * 64:hl * 64 + 64,
                                        n * 512:(n + 1) * 512],
                                    v_sb[:, j - 1, 2 * m + hl, :],
                                    prev_es[hl][:, n * 512:(n + 1) * 512],
                                    start=(j == 1), stop=False,
                                    tile_position=(0, hl * 64),
                                )
                    for task in interleave[ui].get(j, ()):
                        task()
                # epilogue AV for j=15
                for n in range(2):
                    for hl in range(2):
                        nc.tensor.matmul(
                            o_t[hl * 64:hl * 64 + 64, n * 512:(n + 1) * 512],
                            v_sb[:, 15, 2 * m + hl, :],
                            es[hl][:, n * 512:(n + 1) * 512],
                            start=False, stop=True,
                            tile_position=(0, hl * 64),
                        )
                # pull O' off PSUM (frees the merged accumulator for the
                # next unit), then the denominator: D = 1^T esum with a
                # [128,64] all-ones stationary so each head's D lands
                # REPLICATED across its 64 partitions (col-tiled, both
                # heads in one tile) — one full-width DVE reciprocal then
                # yields 1/D with no cross-partition broadcast needed.
                o_cp = np_pool.tile([128, 1024], F16, tag="o_cp", name="o_cp")
                copy16(o_cp[:], o_t[:])
                dps = px.tile([128, 1024], F32, tag="x", name="d_ps")
                for hl in range(2):
                    for n in range(2):
                        nc.tensor.matmul(
                            dps[hl * 64:hl * 64 + 64,
                                n * 512:(n + 1) * 512],
                            ones16[:, :],
                            esum_sb[:, hl, n * 512:(n + 1) * 512],
                            start=True, stop=True,
                            tile_position=(0, hl * 64),
                        )
                rep = st_pool.tile([128, 1024], F32, tag="rep")
                # approx reciprocal (~51 ULP, fp32-only): the exact
                # iterative one runs at ~6 cyc/elem and stalls the DVE 6.5us
                nc.vector.reciprocal_approx_fast(rep[:], dps[:])
                pending.append((m, i0, o_cp, rep))

            # tail: unit 3's normalize + the ih=1 output projection; a dummy
            # matmul group keeps the PE HAM warm across the normalize chain,
            # staging copies alternate scalar/vector
            emit_warm(ps)
            while pending:
                emit_normalize()
            for ct in range(8):
                emit_wo_chunk(1, ct, (px, ps)[ct % 2],
                              copy_engine=("scalar", "vector")[ct % 2])

            ost_pool.release()
            np_pool.release()
            st_pool.release()
            es_pool.release()

    nc.compile()
    return nc


def kernel(x, Wq, Wk, Wv, Wo, bo):
    x = np.asarray(x, dtype=np.float32)
    Wq = np.asarray(Wq, dtype=np.float32)
    Wk = np.asarray(Wk, dtype=np.float32)
    Wv = np.asarray(Wv, dtype=np.float32)
    Wo = np.asarray(Wo, dtype=np.float32)
    bo = np.asarray(bo, dtype=np.float32)

    if "nc" not in _cache:
        _cache["nc"] = _build()
    nc = _cache["nc"]

    xTs = [np.ascontiguousarray(x[b].T).astype(np.float16) for b in range(B)]
    in_maps = []
    for core in range(8):
        b, hg = divmod(core, 4)
        sl = slice(hg * HD, (hg + 1) * HD)
        in_maps.append({
            "xT": xTs[b],
            "wq": np.ascontiguousarray(Wq[:, sl]).astype(np.float16),
            "wk": np.ascontiguousarray(Wk[:, sl]).astype(np.float16),
            "wv": np.ascontiguousarray(Wv[:, sl]).astype(np.float16),
            "wo": np.ascontiguousarray(Wo[sl, :]).astype(np.float16),
        })

    res = run_bass_kernel_spmd(nc, in_maps, core_ids=list(range(8)))
    out = np.empty((B, L, C), dtype=np.float32)
    for b in range(B):
        acc = res.results[4 * b]["outT"]
        for hg in range(1, 4):
            acc = acc + res.results[4 * b + hg]["outT"]
        out[b] = acc.T + bo
    return out
